# revision 30
# baseline (speedup 1.0000x reference)
"""Trainium2 Bass kernel for music-transformer relative attention — v4.

Shapes (hardcoded): x [2, 2048, 1024], 16 heads x 64 dims, MAXLEN == N == 2048.
Sharding: 8 cores = 2 batches x 4 head-groups (4 heads each). Each core computes
its heads' attention and a partial output projection (bf16); host sums the 4
partials per batch and adds the bias.

v4 structure: scores are computed TRANSPOSED in 512-wide query chunks:
S^T[j, i-chunk] = matmul(lhsT=k-block, rhs=q-chunk). The music-transformer
skew uses a 128-row DRAM bounce (contiguous write, (W-1)-strided read); the
srel tiles come back in NATURAL orientation and are added into the scores
PSUM with PE transpose-adds (matmul(ps_slice, lhsT=srel_tile, rhs=ident,
start=False)). exp output A^T feeds the AV matmul directly. Diagonal-zero and
causal -1e9 mask live at fixed right-aligned columns of persistent p_sb
staging tiles (preset once); fully-masked j>i sub-tiles add a constant -1e9
tile. The bounce work for chunk c+1 is emitted as a generator whose quanta
(matmul+copy / write-DMA) are interspersed between attention kb-steps of
chunk c, so no P' instruction ever head-blocks the in-order PE queue for
long. Normalization tails are flushed one unit later; output projection for
chunk c is emitted during chunk c+1.
"""

import sys

sys.path.insert(0, "/opt/trn_rl_repo")

import numpy as np
import ml_dtypes

import concourse.bass as bass
import concourse.tile as tile
from concourse import bacc
from concourse import mybir
from concourse.bass_utils import run_bass_kernel_spmd
from concourse.masks import make_identity

BF = mybir.dt.bfloat16
F8 = mybir.dt.float8e4
F32 = mybir.dt.float32
N = 2048
D = 1024
HD = 64
HPC = 4          # heads per core
DC = HPC * HD    # 256 head dims per core
CH = 512         # query chunk
NCH = N // CH    # 4 chunks
NB = N // 128    # 16 row blocks
PSB_W = 2175     # p_sb staging width = max W = 2048 + 127

_CACHE = {}

DR = mybir.MatmulPerfMode.DoubleRow
import os
USE_DR_TADD = os.environ.get("NO_DR_TADD") != "1"
USE_DR_PP = os.environ.get("NO_DR_PP") != "1"


def _dup2(ap):
    """Stationary/moving AP with a stride-0 plane dim inserted: [P, 2, F]."""
    return bass.AP(ap.tensor, ap.offset, [ap.ap[0], [0, 2]] + ap.ap[1:])


def _planes(ap, stride):
    """AP with an explicit 2-plane dim at `stride` elements: [P, 2, F]."""
    return bass.AP(ap.tensor, ap.offset, [ap.ap[0], [stride, 2]] + ap.ap[1:])


def _W(b):
    return 128 * (b + 1) + 127  # scratch row width for 128-row block b


_BASES = []
_off = 0
for _h in range(HPC):
    for _b in range(NB):
        _BASES.append(_off)
        _off += 128 * _W(_b)
SCRATCH_SZ = _off


def _base(h, b):
    return _BASES[h * NB + b]


def _build_nc():
    nc = bacc.Bacc()
    xT = nc.dram_tensor("xT", [D, N], BF, kind="ExternalInput")
    wqT = nc.dram_tensor("wqT", [D, DC], BF, kind="ExternalInput")
    wkT = nc.dram_tensor("wkT", [D, DC], BF, kind="ExternalInput")
    wvT = nc.dram_tensor("wvT", [D, DC], BF, kind="ExternalInput")
    eT = nc.dram_tensor("eT", [DC, N], F8, kind="ExternalInput")
    wpT = nc.dram_tensor("wpT", [DC, D], BF, kind="ExternalInput")
    outp = nc.dram_tensor("outp", [N, D], BF, kind="ExternalOutput")
    scratch = nc.dram_tensor("scratch", [SCRATCH_SZ], F8)

    from contextlib import ExitStack

    with tile.TileContext(nc) as tc, ExitStack() as ctx:
        pers = ctx.enter_context(tc.tile_pool(name="pers", bufs=1))
        psA = ctx.enter_context(tc.tile_pool(name="psA", bufs=2, space="PSUM"))
        psD = ctx.enter_context(tc.tile_pool(name="psD", bufs=2, space="PSUM"))
        psB = ctx.enter_context(tc.tile_pool(name="psB", bufs=2, space="PSUM"))
        ss = ctx.enter_context(tc.tile_pool(name="ss", bufs=4))
        aa = ctx.enter_context(tc.tile_pool(name="aa", bufs=4))
        oo = ctx.enter_context(tc.tile_pool(name="oo", bufs=4))
        llp = ctx.enter_context(tc.tile_pool(name="llp", bufs=2))

        # ---- persistent SBUF tensors ----
        xt = [pers.tile([128, N], BF, tag=f"xt{i}", name=f"xt{i}") for i in range(8)]
        wq = [pers.tile([128, DC], BF, tag=f"wq{i}", name=f"wq{i}") for i in range(8)]
        wk = [pers.tile([128, DC], BF, tag=f"wk{i}", name=f"wk{i}") for i in range(8)]
        wv = [pers.tile([128, DC], BF, tag=f"wv{i}", name=f"wv{i}") for i in range(8)]
        wp = [pers.tile([128, D], BF, tag=f"wp{i}", name=f"wp{i}") for i in range(2)]
        # etz[h]: fp8 e for head h at partitions [64(h%2), +64); other 64
        # partitions zeroed (kills the cross-head term in the 128-contraction
        # P' DoubleRow); zero tail [N, N+516) hosts the zero plane window
        # (DoubleRow needs even plane strides, so the window starts at N+1
        # when c0a is odd).
        etz = [pers.tile([128, N + 516], F8, tag=f"etz{i}", name=f"etz{i}")
               for i in range(4)]
        qt = [pers.tile([128, N + 1], BF, tag=f"qt{i}", name=f"qt{i}") for i in range(2)]
        # width 2050: DoubleRow stationary reads need an even partition pitch
        qt8 = [pers.tile([128, N + 2], F8, tag=f"qt8{i}", name=f"qt8{i}")
               for i in range(2)]
        kt = [pers.tile([128, N], BF, tag=f"kt{i}", name=f"kt{i}") for i in range(2)]
        vaug = [pers.tile([128, HPC, HD + 1], BF, tag=f"va{i}", name=f"va{i}")
                for i in range(NB)]
        aot = [pers.tile([128, N], BF, tag=f"ao{i}", name=f"ao{i}") for i in range(2)]
        psb = [pers.tile([128, PSB_W], F8, tag=f"psb{i}", name=f"psb{i}")
               for i in range(8)]
        ident = pers.tile([128, 128], BF, tag="ident", name="ident")
        # identz: plane 0 = identity, plane 1 = zeros (zero-plane DoubleRow)
        identz = pers.tile([128, 2, 128], F8, tag="identz", name="identz")
        make_identity(nc, ident[:])
        nc.vector.tensor_copy(identz[:, 0, :], ident[:])
        nc.gpsimd.memset(identz[:, 1, :], 0.0)
        for g in range(2):
            nc.gpsimd.memset(qt[g][:, 0:1], 0.0)
            nc.gpsimd.memset(qt8[g][:, 0:1], 0.0)
        for h in range(4):
            ho, oh = 64 * (h % 2), 64 * (1 - h % 2)
            nc.gpsimd.memset(etz[h][oh:oh + 64, 0:N + 516], 0.0)
            nc.gpsimd.memset(etz[h][ho:ho + 64, N:N + 516], 0.0)
        # p_sb: data right-aligned so the diagonal-zero column sits at
        # PSB_W-128 and the 127 mask columns fill the tail — preset once.
        for i in range(len(psb)):
            nc.gpsimd.memset(psb[i][:, PSB_W - 128:PSB_W - 127], 0.0)
            nc.gpsimd.memset(psb[i][:, PSB_W - 127:PSB_W], -240.0)

        # Input loads, ordered so the kc-outer Q/K projection can start after
        # the first (xt, wq) pair lands.
        for i in range(8):
            nc.sync.dma_start(wq[i][:], wqT[bass.ts(i, 128), :])
            nc.sync.dma_start(xt[i][:], xT[bass.ts(i, 128), :])
            nc.sync.dma_start(wk[i][:], wkT[bass.ts(i, 128), :])
        for i in range(8):
            nc.sync.dma_start(wv[i][:], wvT[bass.ts(i, 128), :])
        for g in range(2):
            nc.sync.dma_start(wp[g][:], wpT[bass.ts(g, 128), :])
        for h in range(4):
            ho = 64 * (h % 2)
            nc.sync.dma_start(etz[h][ho:ho + 64, 0:N],
                              eT[64 * h:64 * h + 64, :])

        # ---- P' bands (srel) -> skewed 128-row scratch blocks
        # Block (h, b): row r holds P'[128b+r-1, m0+p]; data p in [0, valid),
        # diag zero at p=valid, mask -1e9 in (valid, W); valid = 128(b+1)-1,
        # W = valid + 128. Strided (W-1) re-read yields skewed srel rows.
        # Emitted as a generator: each quantum is one matmul+copy or the
        # block's write DMA, so quanta interleave between other PE work.
        psb_state = {"idx": 0}

        def pprime_gen(h, c):
            g, ho = h // 2, 64 * (h % 2)
            for b in range(4 * c, 4 * c + 4):
                W = _W(b)
                valid = W - 128
                p_sb = psb[psb_state["idx"] % len(psb)]
                psb_state["idx"] += 1
                for c0 in range(0, valid, 512):
                    w = min(512, valid - c0)
                    ps = psD.tile([128, 512], F32, tag="pp", name="pp")
                    # P' via zero-plane DoubleRow at 128 contraction rows:
                    # lhsT = q-block duplicated (stride-0 plane); rhs =
                    # (e-chunk, zero-window) planes on the per-head etz whose
                    # other-head partitions are zeroed.
                    c0a = N - valid + c0
                    if USE_DR_PP:
                        # DoubleRow needs even moving width AND even plane
                        # stride; pad cols read the zero window / tail.
                        wp_ = w + (w & 1)
                        s = N - c0a + ((N - c0a) & 1)
                        nc.tensor.matmul(
                            ps[:, 0:wp_],
                            _dup2(qt8[g][0:128, 128 * b:128 * b + 128]),
                            _planes(etz[h][0:128, c0a:c0a + wp_], s),
                            start=True, stop=True, perf_mode=DR)
                    else:
                        nc.tensor.matmul(
                            ps[:, 0:w],
                            qt8[g][0:128, 128 * b:128 * b + 128],
                            etz[h][0:128, c0a:c0a + w],
                            start=True, stop=True)
                    dst = p_sb[:, PSB_W - W + c0:PSB_W - W + c0 + w]
                    # GPSIMD cannot read PSUM on HW: split DVE/ACT 2:1
                    if psb_state["idx"] % 3 == 2:
                        nc.scalar.copy(dst, ps[:, 0:w])
                    else:
                        nc.vector.tensor_copy(dst, ps[:, 0:w])
                    yield
                wr_ap = bass.AP(scratch, _base(h, b), [[W, 128], [1, W]])
                nc.sync.dma_start(wr_ap, p_sb[:, PSB_W - W:PSB_W])
                yield

        def drive(gen, n):
            if gen is None:
                return
            for _ in range(n):
                try:
                    next(gen)
                except StopIteration:
                    break

        bgq = []

        def drive_q(n):
            while n > 0 and bgq:
                try:
                    next(bgq[0])
                    n -= 1
                except StopIteration:
                    bgq.pop(0)

        # ---- projections: QT/KT transposed layout, kc-outer so the first
        # matmul only waits on (xt[0], wq[0]).
        for g in range(2):
            psq = [psA.tile([128, 512], F32, tag="mm", name="mm"),
                   psA.tile([128, 512], F32, tag="mm", name="mm"),
                   psD.tile([128, 512], F32, tag="pp", name="pp"),
                   psD.tile([128, 512], F32, tag="pp", name="pp")]
            for kc in range(8):
                for nchunk in range(4):
                    nc.tensor.matmul(
                        psq[nchunk][:], wq[kc][:, bass.ts(g, 128)],
                        xt[kc][:, bass.ts(nchunk, 512)],
                        start=(kc == 0), stop=(kc == 7))
            for nchunk in range(4):
                nc.scalar.copy(
                    qt[g][:, 1 + nchunk * 512:1 + (nchunk + 1) * 512],
                    psq[nchunk][:])
            for nchunk in range(4):
                nc.gpsimd.tensor_copy(
                    qt8[g][:, 1 + nchunk * 512:1 + (nchunk + 1) * 512],
                    qt[g][:, 1 + nchunk * 512:1 + (nchunk + 1) * 512])
            psk = [psA.tile([128, 512], F32, tag="mm", name="mm"),
                   psA.tile([128, 512], F32, tag="mm", name="mm"),
                   psD.tile([128, 512], F32, tag="pp", name="pp"),
                   psD.tile([128, 512], F32, tag="pp", name="pp")]
            for kc in range(8):
                for nchunk in range(4):
                    nc.tensor.matmul(
                        psk[nchunk][:], wk[kc][:, bass.ts(g, 128)],
                        xt[kc][:, bass.ts(nchunk, 512)],
                        start=(kc == 0), stop=(kc == 7))
            for nchunk in range(4):
                nc.vector.tensor_copy(kt[g][:, bass.ts(nchunk, 512)],
                                      psk[nchunk][:])
            # c=0 bounce for this group's heads hides under remaining proj
            for hh in (2 * g, 2 * g + 1):
                for _ in pprime_gen(hh, 0):
                    pass

        # ---- V natural layout + ones column; chunk-1 bounce generators
        # start draining here so their writes land well before attention
        # reaches chunk 1.
        bg1 = {h: pprime_gen(h, 1) for h in range(HPC)}
        for i in range(NB):
            ps = psA.tile([128, HPC, HD], F32, tag="mm", name="mm")
            for kc in range(8):
                nc.tensor.matmul(
                    ps[:, :, :], xt[kc][:, bass.ts(i, 128)], wv[kc][:],
                    start=(kc == 0), stop=(kc == 7))
            nc.gpsimd.memset(vaug[i][:, :, HD:HD + 1], 1.0)
            if i % 2 == 0:
                nc.vector.tensor_copy(vaug[i][:, :, 0:HD], ps[:, :, :])
            else:
                nc.scalar.copy(vaug[i][:, :, 0:HD], ps[:, :, :])
            drive(bg1[i % HPC], 1)

        # Funnel cross-engine deps into PE's observed clock so no real
        # matmul needs >2 sync waits: dummy [1,1] matmuls reading each
        # phase-boundary tensor, cycling PSUM pools.
        if os.environ.get("NO_F8_FUNNEL") == "1":
            srcs = [qt[0], qt[1], kt[0], kt[1], wp[0], wp[1]]
        else:
            srcs = [etz[0], etz[1], qt[0], qt[1], kt[0], kt[1], wp[0], wp[1],
                    qt8[0], qt8[1], etz[2], etz[3]]
        for i, src in enumerate(srcs):
            if i % 2 == 0:
                ps_d = psA.tile([1, 1], F32, tag="mm", name="mm")
            else:
                ps_d = psB.tile([1, 1], F32, tag="sm", name="sm")
            nc.tensor.matmul(ps_d[0:1, 0:1], src[0:1, 1:2], src[0:1, 1:2],
                             start=True, stop=True)

        # ---- attention: transposed scores + srel transpose-add + exp + AV
        pending = []

        def flush_norm():
            while pending:
                fn = pending.pop(0)
                fn()

        def emit_attn(h, c, bg=None):
            g, ho = h // 2, 64 * (h % 2)
            i0 = CH * c
            nkb = 4 * (c + 1)
            sn = []
            for t in range(4):
                b = 4 * c + t
                W = _W(b)
                cm = 128 * (b + 1)
                s_nat = ss.tile([128, cm], F8, tag=f"sn{t}", name=f"sn{t}")
                rd_ap = bass.AP(scratch, _base(h, b) + 127,
                                [[W - 1, 128], [1, cm]])
                nc.gpsimd.dma_start(s_nat[:], rd_ap)
                sn.append(s_nat)
            ps_o = psB.tile([65, CH], F32, tag="sm", name="sm")
            for kbp in range(nkb // 2):
                # kb pair shares a [128, 2, CH] PSUM tile so ONE exp covers
                # both planes (halves the per-call ACT init overhead).
                psp = psA.tile([128, 2, CH], F32, tag="mm", name="mm")
                offs = []
                for pl in range(2):
                    kb = 2 * kbp + pl
                    # Diagonal j-blocks only need scores for i >= 128*kb:
                    # shrink the computed i-window; sub-diagonal tiles are
                    # skipped entirely.
                    off = max(0, 128 * kb - i0)
                    offs.append(off)
                    nc.tensor.matmul(
                        psp[:, pl, off:CH],
                        kt[g][ho:ho + 64, bass.ts(kb, 128)],
                        qt[g][ho:ho + 64, 1 + i0 + off:1 + i0 + CH],
                        start=True, stop=False)
                    t_lo = max(0, kb - 4 * c)
                    for t in range(t_lo, 4):
                        if USE_DR_TADD:
                            nc.tensor.matmul(
                                psp[:, pl, bass.ts(t, 128)],
                                _dup2(sn[t][:, bass.ts(kb, 128)]),
                                identz[:, :, :], start=False, stop=(t == 3),
                                perf_mode=DR)
                        else:
                            nc.tensor.matmul(
                                psp[:, pl, bass.ts(t, 128)],
                                sn[t][:, bass.ts(kb, 128)],
                                identz[:, 0, :], start=False, stop=(t == 3))
                    drive_q(2 if c else 3)
                # One exp over both planes from the even off; the odd
                # plane's [off_e, off_o) region is garbage that AV never
                # reads.
                off_e = offs[0]
                a2 = aa.tile([128, 2, CH], BF, tag="asb", name="asb")
                nc.scalar.activation(
                    a2[:, :, off_e:CH], psp[:, :, off_e:CH],
                    mybir.ActivationFunctionType.Exp, scale=0.125)
                for pl in range(2):
                    kb = 2 * kbp + pl
                    off = offs[pl]
                    nc.tensor.matmul(
                        ps_o[:, off:CH], vaug[kb][:, h, :],
                        a2[:, pl, off:CH],
                        start=(kb == 0), stop=(kb == nkb - 1))
                if kbp == min(2, nkb // 2 - 1):
                    flush_norm()
                drive_q(2 if c else 3)
            # normalize: aot = ps_o[0:64] * (1 / l); reciprocal reads the
            # l row straight from PSUM; Pool broadcasts it across 64
            # partitions (deferred one unit with the mul).
            linv = llp.tile([1, CH], F32, tag="linv", name="linv")
            nc.vector.reciprocal(linv[:], ps_o[64:65, :])

            def _norm(g=g, ho=ho, i0=i0, ps_o=ps_o, linv=linv):
                lb = llp.tile([64, CH], F32, tag="lb2", name="lb2")
                nc.gpsimd.partition_broadcast(lb[:], linv[:])
                nc.vector.tensor_mul(
                    aot[g][ho:ho + 64, i0:i0 + CH], ps_o[0:64, :], lb[:])
            pending.append(_norm)

        def emit_outproj(c):
            for r0i in range(4 * c, 4 * c + 4):
                for nch in range(2):
                    ps = psA.tile([128, 512], F32, tag="mm", name="mm")
                    for dc in range(2):
                        nc.tensor.matmul(
                            ps[:], aot[dc][:, bass.ts(r0i, 128)],
                            wp[dc][:, bass.ts(nch, 512)],
                            start=(dc == 0), stop=(dc == 1))
                    o_sb = oo.tile([128, 512], BF, tag="osb", name="osb")
                    if nch == 0:
                        nc.vector.tensor_copy(o_sb[:], ps[:])
                    else:
                        nc.scalar.copy(o_sb[:], ps[:])
                    nc.sync.dma_start(
                        outp[bass.ts(r0i, 128), bass.ts(nch, 512)], o_sb[:])

        # Main pipeline: attention of chunk c drives the bounce generator of
        # chunk c+1 between kb-steps; outproj of chunk c-1 follows the first
        # unit of chunk c (after its norms flushed).
        for c in range(NCH):
            for h in range(HPC):
                if c == 0:
                    bgq.append(bg1[h])
                elif c + 1 < NCH:
                    bgq.append(pprime_gen(h, c + 1))
                emit_attn(h, c, None)
                if c > 0 and h == 0:
                    emit_outproj(c - 1)
        flush_norm()
        emit_outproj(NCH - 1)
    nc.compile()
    return nc


def kernel(x, Wq, Wk, Wv, Wp, bp, rel_embed):
    x = np.asarray(x, np.float32)
    bf = ml_dtypes.bfloat16
    f8 = ml_dtypes.float8_e4m3
    if "nc" not in _CACHE:
        _CACHE["nc"] = _build_nc()
    nc = _CACHE["nc"]

    in_maps = []
    for c in range(8):
        b, hg = c // 4, c % 4
        c0 = hg * DC
        in_maps.append({
            "xT": np.ascontiguousarray(x[b].T).astype(bf),
            "wqT": np.ascontiguousarray(np.asarray(Wq)[c0:c0 + DC, :].T).astype(bf),
            "wkT": np.ascontiguousarray(np.asarray(Wk)[c0:c0 + DC, :].T).astype(bf),
            "wvT": np.ascontiguousarray(np.asarray(Wv)[c0:c0 + DC, :].T).astype(bf),
            "eT": np.ascontiguousarray(np.asarray(rel_embed)[:, c0:c0 + DC].T).astype(f8),
            "wpT": np.ascontiguousarray(np.asarray(Wp)[:, c0:c0 + DC].T).astype(bf),
        })
    kw = dict(_CACHE.get("run_kwargs") or {})
    r = run_bass_kernel_spmd(nc, in_maps, list(range(8)), **kw)
    _CACHE["last_result"] = r
    res = r.results
    out = np.zeros((2, N, D), np.float32)
    for c in range(8):
        out[c // 4] += np.asarray(res[c]["outp"], np.float32)
    out += np.asarray(bp, np.float32)
    return out



# revision 47
# speedup vs baseline: 1.1315x; 1.1315x over previous
"""Trainium2 Bass kernel for music-transformer relative attention — v4.

Shapes (hardcoded): x [2, 2048, 1024], 16 heads x 64 dims, MAXLEN == N == 2048.
Sharding: 8 cores = 2 batches x 4 head-groups (4 heads each). Each core computes
its heads' attention and a partial output projection (bf16); host sums the 4
partials per batch and adds the bias.

v4 structure: scores are computed TRANSPOSED in 512-wide query chunks:
S^T[j, i-chunk] = matmul(lhsT=k-block, rhs=q-chunk). The music-transformer
skew uses a 128-row DRAM bounce (contiguous write, (W-1)-strided read); the
srel tiles come back in NATURAL orientation and are added into the scores
PSUM with PE transpose-adds (matmul(ps_slice, lhsT=srel_tile, rhs=ident,
start=False)). exp output A^T feeds the AV matmul directly. Diagonal-zero and
causal -1e9 mask live at fixed right-aligned columns of persistent p_sb
staging tiles (preset once); fully-masked j>i sub-tiles add a constant -1e9
tile. The bounce work for chunk c+1 is emitted as a generator whose quanta
(matmul+copy / write-DMA) are interspersed between attention kb-steps of
chunk c, so no P' instruction ever head-blocks the in-order PE queue for
long. Normalization tails are flushed one unit later; output projection for
chunk c is emitted during chunk c+1.
"""

import sys

sys.path.insert(0, "/opt/trn_rl_repo")

import numpy as np
import ml_dtypes

import concourse.bass as bass
import concourse.tile as tile
from concourse import bacc
from concourse import mybir
from concourse.bass_utils import run_bass_kernel_spmd
from concourse.masks import make_identity

BF = mybir.dt.bfloat16
F8 = mybir.dt.float8e4
F32 = mybir.dt.float32
N = 2048
D = 1024
HD = 64
HPC = 4          # heads per core
DC = HPC * HD    # 256 head dims per core
CH = 512         # query chunk
NCH = N // CH    # 4 chunks
NB = N // 128    # 16 row blocks
PSB_W = 2175     # p_sb staging width = max W = 2048 + 127

_CACHE = {}

DR = mybir.MatmulPerfMode.DoubleRow
import os
USE_DR_TADD = os.environ.get("NO_DR_TADD") != "1"
USE_DR_PP = os.environ.get("NO_DR_PP") != "1"


def _dup2(ap):
    """Stationary/moving AP with a stride-0 plane dim inserted: [P, 2, F]."""
    return bass.AP(ap.tensor, ap.offset, [ap.ap[0], [0, 2]] + ap.ap[1:])


def _planes(ap, stride):
    """AP with an explicit 2-plane dim at `stride` elements: [P, 2, F]."""
    return bass.AP(ap.tensor, ap.offset, [ap.ap[0], [stride, 2]] + ap.ap[1:])


def _W(b):
    return 128 * (b + 1) + 127  # scratch row width for 128-row block b


_BASES = []
_off = 0
for _h in range(HPC):
    for _b in range(NB):
        _BASES.append(_off)
        _off += 128 * _W(_b)
SCRATCH_SZ = _off


def _base(h, b):
    return _BASES[h * NB + b]


def _build_nc():
    nc = bacc.Bacc()
    # x and the QKV weights ship as fp8 hi/lo residual planes (the weights
    # pre-scaled by 64 on the host; the PSUM->SBUF copies scale by 1/64).
    # Layout: [kc-pair t][partition][kc j][hi/lo s][cols].
    xT8 = nc.dram_tensor("xT8", [4, 128, 2, 2, N], F8, kind="ExternalInput")
    wq8T = nc.dram_tensor("wq8T", [4, 128, 2, 2, DC], F8, kind="ExternalInput")
    wk8T = nc.dram_tensor("wk8T", [4, 128, 2, 2, DC], F8, kind="ExternalInput")
    wv8T = nc.dram_tensor("wv8T", [4, 128, 2, 2, DC], F8, kind="ExternalInput")
    eT = nc.dram_tensor("eT", [DC, N], F8, kind="ExternalInput")
    wpT = nc.dram_tensor("wpT", [DC, D], BF, kind="ExternalInput")
    outp = nc.dram_tensor("outp", [N, D], BF, kind="ExternalOutput")
    scratch = nc.dram_tensor("scratch", [SCRATCH_SZ], F8)

    from contextlib import ExitStack

    with tile.TileContext(nc) as tc, ExitStack() as ctx:
        pers = ctx.enter_context(tc.tile_pool(name="pers", bufs=1))
        psA = ctx.enter_context(tc.tile_pool(name="psA", bufs=4, space="PSUM"))
        psD = ctx.enter_context(tc.tile_pool(name="psD", bufs=2, space="PSUM"))
        psB = ctx.enter_context(tc.tile_pool(name="psB", bufs=2, space="PSUM"))
        ss = ctx.enter_context(tc.tile_pool(name="ss", bufs=4))
        aa = ctx.enter_context(tc.tile_pool(name="aa", bufs=6))
        oo = ctx.enter_context(tc.tile_pool(name="oo", bufs=4))
        llp = ctx.enter_context(tc.tile_pool(name="llp", bufs=2))

        # ---- persistent SBUF tensors ----
        xt2 = [pers.tile([128, 2, 2, N], F8, tag=f"xt{i}", name=f"xt{i}")
               for i in range(4)]
        wq8 = [pers.tile([128, 2, 2, DC], F8, tag=f"wq{i}", name=f"wq{i}")
               for i in range(4)]
        wk8 = [pers.tile([128, 2, 2, DC], F8, tag=f"wk{i}", name=f"wk{i}")
               for i in range(4)]
        wv8 = [pers.tile([128, 2, 2, DC], F8, tag=f"wv{i}", name=f"wv{i}")
               for i in range(4)]
        wp = [pers.tile([128, D], BF, tag=f"wp{i}", name=f"wp{i}") for i in range(2)]
        # etz[h]: fp8 e for head h at partitions [64(h%2), +64); other 64
        # partitions zeroed (kills the cross-head term in the 128-contraction
        # P' DoubleRow); zero tail [N, N+516) hosts the zero plane window
        # (DoubleRow needs even plane strides, so the window starts at N+1
        # when c0a is odd).
        etz = [pers.tile([128, N + 516], F8, tag=f"etz{i}", name=f"etz{i}")
               for i in range(4)]
        qt = [pers.tile([128, N + 1], BF, tag=f"qt{i}", name=f"qt{i}") for i in range(2)]
        # width 2050: DoubleRow stationary reads need an even partition pitch
        qt8 = [pers.tile([128, N + 2], F8, tag=f"qt8{i}", name=f"qt8{i}")
               for i in range(2)]
        kt = [pers.tile([128, N], BF, tag=f"kt{i}", name=f"kt{i}") for i in range(2)]
        vaug = [pers.tile([128, HPC, HD + 1], BF, tag=f"va{i}", name=f"va{i}")
                for i in range(NB)]
        aot = [pers.tile([128, N], BF, tag=f"ao{i}", name=f"ao{i}") for i in range(2)]
        psb = [pers.tile([128, PSB_W], F8, tag=f"psb{i}", name=f"psb{i}")
               for i in range(8)]
        ident = pers.tile([128, 128], BF, tag="ident", name="ident")
        # identz: plane 0 = identity, plane 1 = zeros (zero-plane DoubleRow)
        identz = pers.tile([128, 2, 128], F8, tag="identz", name="identz")
        ones = pers.tile([1, 64], F32, tag="ones", name="ones")
        make_identity(nc, ident[:])
        nc.vector.tensor_copy(identz[:, 0, :], ident[:])
        nc.gpsimd.memset(identz[:, 1, :], 0.0)
        nc.gpsimd.memset(ones[:], 1.0)
        for g in range(2):
            nc.gpsimd.memset(qt[g][:, 0:1], 0.0)
            nc.gpsimd.memset(qt8[g][:, 0:1], 0.0)
        for h in range(4):
            ho, oh = 64 * (h % 2), 64 * (1 - h % 2)
            nc.gpsimd.memset(etz[h][oh:oh + 64, 0:N + 516], 0.0)
            nc.gpsimd.memset(etz[h][ho:ho + 64, N:N + 516], 0.0)
        # p_sb: data right-aligned so the diagonal-zero column sits at
        # PSB_W-128 and the 127 mask columns fill the tail — preset once.
        for i in range(len(psb)):
            nc.gpsimd.memset(psb[i][:, PSB_W - 128:PSB_W - 127], 0.0)
            nc.gpsimd.memset(psb[i][:, PSB_W - 127:PSB_W], -240.0)

        # Input loads, ordered so the t-outer Q/K projection can start after
        # wq8[t] plus the x planes it consumes first (hi before lo).
        for t in range(4):
            nc.sync.dma_start(wq8[t][:], wq8T[t, :, :, :, :])
            for (j, s) in ((0, 0), (1, 0), (0, 1), (1, 1)):
                nc.sync.dma_start(xt2[t][:, j, s, :], xT8[t, :, j, s, :])
            nc.sync.dma_start(wk8[t][:], wk8T[t, :, :, :, :])
        for t in range(4):
            nc.sync.dma_start(wv8[t][:], wv8T[t, :, :, :, :])
        for g in range(2):
            nc.sync.dma_start(wp[g][:], wpT[bass.ts(g, 128), :])
        for h in range(4):
            ho = 64 * (h % 2)
            nc.sync.dma_start(etz[h][ho:ho + 64, 0:N],
                              eT[64 * h:64 * h + 64, :])

        # ---- P' bands (srel) -> skewed 128-row scratch blocks
        # Block (h, b): row r holds P'[128b+r-1, m0+p]; data p in [0, valid),
        # diag zero at p=valid, mask -1e9 in (valid, W); valid = 128(b+1)-1,
        # W = valid + 128. Strided (W-1) re-read yields skewed srel rows.
        # Emitted as a generator: each quantum is one matmul+copy or the
        # block's write DMA, so quanta interleave between other PE work.
        psb_state = {"idx": 0}

        def pprime_gen(h, c):
            g, ho = h // 2, 64 * (h % 2)
            for b in range(4 * c, 4 * c + 4):
                W = _W(b)
                valid = W - 128
                p_sb = psb[psb_state["idx"] % len(psb)]
                psb_state["idx"] += 1
                for c0 in range(0, valid, 512):
                    w = min(512, valid - c0)
                    ps = psD.tile([128, 512], F32, tag="pp", name="pp")
                    # P' via zero-plane DoubleRow at 128 contraction rows:
                    # lhsT = q-block duplicated (stride-0 plane); rhs =
                    # (e-chunk, zero-window) planes on the per-head etz whose
                    # other-head partitions are zeroed.
                    c0a = N - valid + c0
                    if USE_DR_PP:
                        # DoubleRow needs even moving width AND even plane
                        # stride; pad cols read the zero window / tail.
                        wp_ = w + (w & 1)
                        s = N - c0a + ((N - c0a) & 1)
                        nc.tensor.matmul(
                            ps[:, 0:wp_],
                            _dup2(qt8[g][0:128, 128 * b:128 * b + 128]),
                            _planes(etz[h][0:128, c0a:c0a + wp_], s),
                            start=True, stop=True, perf_mode=DR)
                    else:
                        nc.tensor.matmul(
                            ps[:, 0:w],
                            qt8[g][0:128, 128 * b:128 * b + 128],
                            etz[h][0:128, c0a:c0a + w],
                            start=True, stop=True)
                    dst = p_sb[:, PSB_W - W + c0:PSB_W - W + c0 + w]
                    # GPSIMD cannot read PSUM on HW: split DVE/ACT 2:1
                    if psb_state["idx"] % 3 == 2:
                        nc.scalar.copy(dst, ps[:, 0:w])
                    else:
                        nc.vector.tensor_copy(dst, ps[:, 0:w])
                    yield
                wr_ap = bass.AP(scratch, _base(h, b), [[W, 128], [1, W]])
                nc.sync.dma_start(wr_ap, p_sb[:, PSB_W - W:PSB_W])
                yield

        def drive(gen, n):
            if gen is None:
                return
            for _ in range(n):
                try:
                    next(gen)
                except StopIteration:
                    break

        bgq = []

        def drive_q(n):
            while n > 0 and bgq:
                try:
                    next(bgq[0])
                    n -= 1
                except StopIteration:
                    bgq.pop(0)

        # ---- projections: 3-plane hi/lo fp8 DoubleRow, t-outer. Per kc-pair
        # t: M1 = (w0hi,w0lo)x(x0hi dup), M3 = (w1hi,w1lo)x(x1hi dup),
        # M2 = (w0hi,w1hi)x(x0lo,x1lo) — together w.T x at bf16-level
        # precision for 75% of the bf16 PE cost. Copies scale by 1/64 to
        # undo the host-side weight scaling.
        def proj_mms(pst, wsb, t, cols, mov):
            first = (t == 0)
            last = (t == 3)
            nc.tensor.matmul(
                pst, wsb[t][:, 0, :, cols], _dup2(mov(t, 0, 0)),
                start=first, stop=False, perf_mode=DR)
            nc.tensor.matmul(
                pst, wsb[t][:, 1, :, cols], _dup2(mov(t, 1, 0)),
                start=False, stop=False, perf_mode=DR)
            nc.tensor.matmul(
                pst, wsb[t][:, :, 0, cols], mov(t, None, 1),
                start=False, stop=last, perf_mode=DR)

        for g in range(2):
            gcols = slice(128 * g, 128 * (g + 1))
            psq = [psA.tile([128, 512], F32, tag="mm", name="mm"),
                   psA.tile([128, 512], F32, tag="mm", name="mm"),
                   psD.tile([128, 512], F32, tag="pp", name="pp"),
                   psD.tile([128, 512], F32, tag="pp", name="pp")]
            for t in range(4):
                for nchunk in range(4):
                    def mov(tt, j, s, nchunk=nchunk):
                        cs = slice(512 * nchunk, 512 * (nchunk + 1))
                        if j is None:
                            return xt2[tt][:, :, s, cs]
                        return xt2[tt][:, j, s, cs]
                    proj_mms(psq[nchunk][:], wq8, t, gcols, mov)
            for nchunk in range(4):
                nc.scalar.activation(
                    qt[g][:, 1 + nchunk * 512:1 + (nchunk + 1) * 512],
                    psq[nchunk][:], mybir.ActivationFunctionType.Copy,
                    scale=1.0 / 64)
            for nchunk in range(4):
                nc.gpsimd.tensor_copy(
                    qt8[g][:, 1 + nchunk * 512:1 + (nchunk + 1) * 512],
                    qt[g][:, 1 + nchunk * 512:1 + (nchunk + 1) * 512])
            psk = [psA.tile([128, 512], F32, tag="mm", name="mm"),
                   psA.tile([128, 512], F32, tag="mm", name="mm"),
                   psD.tile([128, 512], F32, tag="pp", name="pp"),
                   psD.tile([128, 512], F32, tag="pp", name="pp")]
            for t in range(4):
                for nchunk in range(4):
                    def mov(tt, j, s, nchunk=nchunk):
                        cs = slice(512 * nchunk, 512 * (nchunk + 1))
                        if j is None:
                            return xt2[tt][:, :, s, cs]
                        return xt2[tt][:, j, s, cs]
                    proj_mms(psk[nchunk][:], wk8, t, gcols, mov)
            for nchunk in range(4):
                nc.vector.tensor_scalar_mul(
                    kt[g][:, bass.ts(nchunk, 512)], psk[nchunk][:], 1.0 / 64)
            # c=0 bounce for this group's heads hides under remaining proj
            for hh in (2 * g, 2 * g + 1):
                for _ in pprime_gen(hh, 0):
                    pass

        # ---- V natural layout + ones column; chunk-1 bounce generators
        # start draining here so their writes land well before attention
        # reaches chunk 1. V uses the same 3-plane trick with x stationary.
        bg1 = {h: pprime_gen(h, 1) for h in range(HPC)}
        for i in range(NB):
            ps = psA.tile([128, HPC, HD], F32, tag="mm", name="mm")
            ib = slice(128 * i, 128 * (i + 1))
            for t in range(4):
                nc.tensor.matmul(
                    ps[:, :, :], xt2[t][:, 0, :, ib], _dup2(wv8[t][:, 0, 0, :]),
                    start=(t == 0), stop=False, perf_mode=DR)
                nc.tensor.matmul(
                    ps[:, :, :], xt2[t][:, 1, :, ib], _dup2(wv8[t][:, 1, 0, :]),
                    start=False, stop=False, perf_mode=DR)
                nc.tensor.matmul(
                    ps[:, :, :], xt2[t][:, :, 0, ib], wv8[t][:, :, 1, :],
                    start=False, stop=(t == 3), perf_mode=DR)
            nc.gpsimd.memset(vaug[i][:, :, HD:HD + 1], 1.0)
            if i % 2 == 0:
                nc.vector.tensor_scalar_mul(vaug[i][:, :, 0:HD], ps[:, :, :],
                                            1.0 / 64)
            else:
                nc.scalar.activation(vaug[i][:, :, 0:HD], ps[:, :, :],
                                     mybir.ActivationFunctionType.Copy,
                                     scale=1.0 / 64)
            drive(bg1[i % HPC], 1)

        # Funnel cross-engine deps into PE's observed clock so no real
        # matmul needs >2 sync waits: dummy [1,1] matmuls reading each
        # phase-boundary tensor, cycling PSUM pools.
        if os.environ.get("NO_F8_FUNNEL") == "1":
            srcs = [qt[0], qt[1], kt[0], kt[1], wp[0], wp[1]]
        else:
            srcs = [etz[0], etz[1], qt[0], qt[1], kt[0], kt[1], wp[0], wp[1],
                    qt8[0], qt8[1], etz[2], etz[3]]
        for i, src in enumerate(srcs):
            if i % 2 == 0:
                ps_d = psA.tile([1, 1], F32, tag="mm", name="mm")
            else:
                ps_d = psB.tile([1, 1], F32, tag="sm", name="sm")
            nc.tensor.matmul(ps_d[0:1, 0:1], src[0:1, 1:2], src[0:1, 1:2],
                             start=True, stop=True)

        # ---- attention: transposed scores + srel transpose-add + exp + AV
        pending = []

        def flush_norm():
            while pending:
                fn = pending.pop(0)
                fn()

        def emit_attn(h, c, bg=None):
            g, ho = h // 2, 64 * (h % 2)
            i0 = CH * c
            nkb = 4 * (c + 1)
            sn = []
            for t in range(4):
                b = 4 * c + t
                W = _W(b)
                cm = 128 * (b + 1)
                s_nat = ss.tile([128, cm], F8, tag=f"sn{t}", name=f"sn{t}")
                rd_ap = bass.AP(scratch, _base(h, b) + 127,
                                [[W - 1, 128], [1, cm]])
                nc.gpsimd.dma_start(s_nat[:], rd_ap)
                sn.append(s_nat)
            ps_o = psB.tile([65, CH], F32, tag="sm", name="sm")
            # Two-step software pipeline: AV(kb-2) is emitted AFTER
            # score/tadds(kb), so exp(kb-2) has ~2 kb of PE work to hide
            # behind — the PE never sits waiting on the exp.
            pend_av = []

            def emit_av(kb, off, a_sb):
                nc.tensor.matmul(
                    ps_o[:, off:CH], vaug[kb][:, h, :], a_sb[:, off:CH],
                    start=(kb == 0), stop=(kb == nkb - 1))

            for kb in range(nkb):
                # Diagonal j-blocks only need scores for i >= 128*kb: shrink
                # the computed i-window; sub-diagonal tiles are skipped.
                off = max(0, 128 * kb - i0)
                ps = psA.tile([128, CH], F32, tag="mm", name="mm")
                nc.tensor.matmul(
                    ps[:, off:CH],
                    kt[g][ho:ho + 64, bass.ts(kb, 128)],
                    qt[g][ho:ho + 64, 1 + i0 + off:1 + i0 + CH],
                    start=True, stop=False)
                t_lo = max(0, kb - 4 * c)
                for t in range(t_lo, 4):
                    if USE_DR_TADD:
                        nc.tensor.matmul(
                            ps[:, bass.ts(t, 128)],
                            _dup2(sn[t][:, bass.ts(kb, 128)]),
                            identz[:, :, :], start=False, stop=(t == 3),
                            perf_mode=DR)
                    else:
                        nc.tensor.matmul(
                            ps[:, bass.ts(t, 128)], sn[t][:, bass.ts(kb, 128)],
                            identz[:, 0, :], start=False, stop=(t == 3))
                if len(pend_av) >= 2:
                    emit_av(*pend_av.pop(0))
                a_sb = aa.tile([128, CH], BF, tag="asb", name="asb")
                nc.scalar.activation(
                    a_sb[:, off:CH], ps[:, off:CH],
                    mybir.ActivationFunctionType.Exp, scale=0.125)
                pend_av.append((kb, off, a_sb))
                if kb == min(5, nkb - 1):
                    flush_norm()
                drive_q(2 if c else 3)
            for av in pend_av:
                emit_av(*av)
            # normalize: aot = ps_o[0:64] * (1 / l); reciprocal reads the
            # l row straight from PSUM; Pool broadcasts it across 64
            # partitions (deferred one unit with the mul).
            linv = llp.tile([1, CH], F32, tag="linv", name="linv")
            nc.vector.reciprocal(linv[:], ps_o[64:65, :])

            def _norm(g=g, ho=ho, i0=i0, ps_o=ps_o, linv=linv):
                ps_l = psA.tile([64, CH], F32, tag="mm", name="mm")
                nc.tensor.matmul(ps_l[:], ones[:], linv[:], start=True,
                                 stop=True)
                lb = llp.tile([64, CH], F32, tag="lb2", name="lb2")
                nc.scalar.copy(lb[:], ps_l[:])
                nc.vector.tensor_mul(
                    aot[g][ho:ho + 64, i0:i0 + CH], ps_o[0:64, :], lb[:])
            pending.append(_norm)

        def emit_outproj(c):
            for r0i in range(4 * c, 4 * c + 4):
                for nch in range(2):
                    ps = psA.tile([128, 512], F32, tag="mm", name="mm")
                    for dc in range(2):
                        nc.tensor.matmul(
                            ps[:], aot[dc][:, bass.ts(r0i, 128)],
                            wp[dc][:, bass.ts(nch, 512)],
                            start=(dc == 0), stop=(dc == 1))
                    o_sb = oo.tile([128, 512], BF, tag="osb", name="osb")
                    if nch == 0:
                        nc.vector.tensor_copy(o_sb[:], ps[:])
                    else:
                        nc.scalar.copy(o_sb[:], ps[:])
                    nc.sync.dma_start(
                        outp[bass.ts(r0i, 128), bass.ts(nch, 512)], o_sb[:])

        # Main pipeline: attention of chunk c drives the bounce generator of
        # chunk c+1 between kb-steps; outproj of chunk c-1 follows the first
        # unit of chunk c (after its norms flushed).
        for c in range(NCH):
            for h in range(HPC):
                if c == 0:
                    bgq.append(bg1[h])
                elif c + 1 < NCH:
                    bgq.append(pprime_gen(h, c + 1))
                emit_attn(h, c, None)
                if c > 0 and h == 0:
                    emit_outproj(c - 1)
        flush_norm()
        emit_outproj(NCH - 1)
    nc.compile()
    return nc


def kernel(x, Wq, Wk, Wv, Wp, bp, rel_embed):
    x = np.asarray(x, np.float32)
    bf = ml_dtypes.bfloat16
    f8 = ml_dtypes.float8_e4m3
    if "nc" not in _CACHE:
        _CACHE["nc"] = _build_nc()
    nc = _CACHE["nc"]

    def hilo8(a):
        # [D, C] -> [4, 128, 2, 2, C] fp8 hi/lo planes over kc-pairs
        hi = a.astype(f8)
        lo = (a - hi.astype(np.float32)).astype(f8)
        out = np.empty((4, 128, 2, 2) + a.shape[1:], dtype=f8)
        for t in range(4):
            for j in range(2):
                blk = slice((2 * t + j) * 128, (2 * t + j + 1) * 128)
                out[t, :, j, 0] = hi[blk]
                out[t, :, j, 1] = lo[blk]
        return out

    xb = [hilo8(np.ascontiguousarray(x[b].T)) for b in range(2)]
    wq_s, wk_s, wv_s = (64.0 * np.asarray(w, np.float32) for w in (Wq, Wk, Wv))
    in_maps = []
    for c in range(8):
        b, hg = c // 4, c % 4
        c0 = hg * DC
        in_maps.append({
            "xT8": xb[b],
            "wq8T": hilo8(np.ascontiguousarray(wq_s[c0:c0 + DC, :].T)),
            "wk8T": hilo8(np.ascontiguousarray(wk_s[c0:c0 + DC, :].T)),
            "wv8T": hilo8(np.ascontiguousarray(wv_s[c0:c0 + DC, :].T)),
            "eT": np.ascontiguousarray(np.asarray(rel_embed)[:, c0:c0 + DC].T).astype(f8),
            "wpT": np.ascontiguousarray(np.asarray(Wp)[:, c0:c0 + DC].T).astype(bf),
        })
    kw = dict(_CACHE.get("run_kwargs") or {})
    r = run_bass_kernel_spmd(nc, in_maps, list(range(8)), **kw)
    _CACHE["last_result"] = r
    res = r.results
    out = np.zeros((2, N, D), np.float32)
    for c in range(8):
        out[c // 4] += np.asarray(res[c]["outp"], np.float32)
    out += np.asarray(bp, np.float32)
    return out

